# revision 15
# baseline (speedup 1.0000x reference)
"""Trainium2 Bass kernel for nn_Net_81707457839588.

10-qubit batched statevector circuit + FC head, factorized as:
  psi_in  = m * phase          (product state after RX embedding; m real >=0... real)
  psi_out = U' @ m             (U' = circuit unitary with phases folded, shared
                                across batch since circuit weights are shared)
  out     = (re^2 + im^2) @ W2 + fc_b    (W2 = SIGNS @ fc_w.T folded)

Device work per core (512 of 4096 samples, pure batch-parallel):
  reT/imT (1024n x 512b) = sum_k U'T[k,n] * mT[k,b]   -> 128 PE matmuls (f32r)
  probsT = reT^2 + imT^2                              -> DVE
  outT (4 x 512) = W2.T @ probsT                      -> 8 PE matmuls
Host: builds U' (tiny circuit, 210 two-level gates on 1024x1024), m (kron
chain), shards batch across 8 cores, gathers, adds fc_b.
"""

import os
import numpy as np

N = 10
L = 10
BATCH = 4096
N_CORES = 8
BC = BATCH // N_CORES  # 512 samples per core
DIM = 2 ** N  # 1024
P = 128
KT = DIM // P  # 8 contraction tiles
NT = DIM // P  # 8 output n tiles


# ---------------------------------------------------------------- host math --

def _apply_gate_np(state, g, w):
    st = np.moveaxis(state, w + 1, 1)
    shp = st.shape
    out = np.einsum('ij,bjm->bim', g, st.reshape(shp[0], 2, -1)).reshape(shp)
    return np.moveaxis(out, 1, w + 1)


def _cnot_np(state, c, t):
    s0 = np.take(state, 0, axis=c + 1)
    s1 = np.take(state, 1, axis=c + 1)
    t_ax = t + 1 if t < c else t
    s1 = np.flip(s1, axis=t_ax)
    return np.stack([s0, s1], axis=c + 1)


def _build_UT(weights):
    """UT[j,k] = <k|U|j> for the StronglyEntanglingLayers circuit (no embedding)."""
    state = np.zeros((DIM, DIM), dtype=np.complex64)
    state[np.arange(DIM), np.arange(DIM)] = 1.0
    state = state.reshape((DIM,) + (2,) * N)
    for l in range(L):
        phi, theta, omega = weights[l, :, 0], weights[l, :, 1], weights[l, :, 2]
        ct, st_ = np.cos(0.5 * theta), np.sin(0.5 * theta)
        ep = np.exp(-0.5j * (phi + omega))
        em = np.exp(0.5j * (phi - omega))
        for w in range(N):
            g = np.array([
                [ep[w] * ct[w], -em[w] * st_[w]],
                [np.conj(em[w]) * st_[w], np.conj(ep[w]) * ct[w]],
            ], dtype=np.complex64)
            state = _apply_gate_np(state, g, w)
        r = l % (N - 1) + 1
        for w in range(N):
            state = _cnot_np(state, w, (w + r) % N)
    return np.ascontiguousarray(state.reshape(DIM, DIM))


def _pauli_z_signs():
    idx = np.arange(DIM)
    bits = (idx[:, None] >> (N - 1 - np.arange(N))) & 1
    return (1.0 - 2.0 * bits).astype(np.float32)


def _host_factors(inputs, weights, fc_w):
    x = np.maximum(np.asarray(inputs, np.float32), 0.0)
    half = 0.5 * x
    c, s = np.cos(half), np.sin(half)
    B = x.shape[0]
    m = np.ones((B, 1), dtype=np.float32)
    for w in range(N):
        v = np.stack([c[:, w], s[:, w]], axis=-1)
        m = (m[:, :, None] * v[:, None, :]).reshape(B, -1)
    k = np.arange(DIM)
    pc = np.zeros(DIM, dtype=np.int64)
    for w in range(N):
        pc += (k >> w) & 1
    ph = ((-1j) ** (pc % 4)).astype(np.complex64)

    UpT = _build_UT(np.asarray(weights, np.float32)) * ph[:, None]
    UrT = np.ascontiguousarray(UpT.real.astype(np.float32))
    UiT = np.ascontiguousarray(UpT.imag.astype(np.float32))
    W2 = _pauli_z_signs() @ np.asarray(fc_w, np.float32).T  # (1024, 4)
    return m, UrT, UiT, W2


# ------------------------------------------------------------- bass kernel --

_NC_CACHE = {}


def _build_bass():
    key = os.environ.get("KERNEL_MM_DTYPE", "fp16")
    if key in _NC_CACHE:
        return _NC_CACHE[key]
    import concourse.bacc as bacc
    import concourse.mybir as mybir
    from concourse.tile import TileContext

    f32 = mybir.dt.float32
    mm_dt = {"f32r": mybir.dt.float32r, "f32": mybir.dt.float32,
             "fp16": mybir.dt.float16, "bf16": mybir.dt.bfloat16}[key]
    small_dt = mm_dt if key in ("fp16", "bf16") else f32

    nc = bacc.Bacc()
    mt_d = nc.dram_tensor("mt", (DIM, BC), mm_dt, kind="ExternalInput")
    urt_d = nc.dram_tensor("urt", (DIM, DIM), mm_dt, kind="ExternalInput")
    uit_d = nc.dram_tensor("uit", (DIM, DIM), mm_dt, kind="ExternalInput")
    w2_d = nc.dram_tensor("w2", (P, NT * 4), small_dt, kind="ExternalInput")
    out_d = nc.dram_tensor("outT", (4, BC), f32, kind="ExternalOutput")

    H = NT // 2  # half of the n-tiles per pipeline phase

    with TileContext(nc) as tc:
        with tc.tile_pool(name="wts", bufs=1) as wpool, \
             tc.tile_pool(name="acts", bufs=1) as apool, \
             tc.tile_pool(name="sq", bufs=8) as sqpool, \
             tc.tile_pool(name="ps", bufs=7, space="PSUM") as pspool, \
             tc.tile_pool(name="pso", bufs=1, space="PSUM") as psopool, \
             tc.tile_pool(name="outp", bufs=1) as opool:

            # ---- PE warm-up: keep the HAM clock-gate busy on dummy data so
            # the real matmuls start at 2.4 GHz instead of 1.2 GHz.
            dummy = apool.tile([P, 256], mm_dt, name="dummy", tag="dummy")
            nc.vector.memset(dummy[:], 0)
            ps_w = pspool.tile([P, 256], f32, name="psw", tag="ps")
            for i in range(16):
                nc.tensor.matmul(ps_w[:], dummy[:, 0:P], dummy[:],
                                 start=(i == 0), stop=(i == 15))
            scrap = opool.tile([P, 8], f32, name="scrap", tag="scrap")
            nc.vector.tensor_copy(scrap[:], ps_w[:, 0:8])

            # ---- input loads: all on sync, interleaved (mt,ur) pairs then ui
            # (serialized ~0.65us triggers double as just-in-time pacing).
            mt, ur, ui = [], [], []
            for kt in range(KT):
                t = apool.tile([P, BC], mm_dt, name=f"mt{kt}", tag=f"mt{kt}")
                nc.sync.dma_start(t[:], mt_d[kt * P:(kt + 1) * P, :])
                mt.append(t)
                t = wpool.tile([P, DIM], mm_dt, name=f"ur{kt}", tag=f"ur{kt}")
                nc.sync.dma_start(t[:], urt_d[kt * P:(kt + 1) * P, :])
                ur.append(t)
            w2t = wpool.tile([P, NT * 4], small_dt, name="w2t", tag="w2")
            nc.sync.dma_start(w2t[:], w2_d[:, :])
            for kt in range(KT):
                t = wpool.tile([P, DIM], mm_dt, name=f"ui{kt}", tag=f"ui{kt}")
                nc.sync.dma_start(t[:], uit_d[kt * P:(kt + 1) * P, :])
                ui.append(t)

            ps_out = psopool.tile([4, BC], f32, name="psout", tag="pso")
            n_proj = [0]

            def proj(sq, nt):
                nc.tensor.matmul(
                    ps_out[:], w2t[:, nt * 4:(nt + 1) * 4], sq[:],
                    start=(n_proj[0] == 0), stop=(n_proj[0] == 2 * NT - 1))
                n_proj[0] += 1

            def mk_sq_act(ps, nt, i):
                t = sqpool.tile([P, BC], small_dt, name=f"sqa{i}_{nt}", tag="sqa")
                nc.scalar.square(t[:], ps[:])
                proj(t, nt)

            def mk_sq_dve(ps, nt):
                c = sqpool.tile([P, BC], f32, name=f"c{nt}", tag="imc")
                nc.vector.tensor_copy(c[:], ps[:])
                t = sqpool.tile([P, BC], small_dt, name=f"sqd{nt}", tag="sqd")
                nc.vector.tensor_mul(t[:], c[:], c[:])
                proj(t, nt)

            # ---- half-phases: re[0:4], re[4:8], im[0:4], im[4:8]; each half
            # holds 4 PSUM banks, squares of the previous half overlap the
            # matmuls of the next and release its banks.
            ps_re, ps_im = {}, {}
            for h in range(2):
                nts = range(h * H, (h + 1) * H)
                for nt in nts:
                    ps_re[nt] = pspool.tile([P, BC], f32, name=f"psre{nt}", tag="ps")
                for kt in range(KT):
                    for nt in nts:
                        nc.tensor.matmul(
                            ps_re[nt][:], ur[kt][:, nt * P:(nt + 1) * P], mt[kt][:],
                            start=(kt == 0), stop=(kt == KT - 1))
                for nt in nts:
                    mk_sq_act(ps_re[nt], nt, 0)
            for h in range(2):
                nts = range(h * H, (h + 1) * H)
                for nt in nts:
                    ps_im[nt] = pspool.tile([P, BC], f32, name=f"psim{nt}", tag="ps")
                for kt in range(KT):
                    for nt in nts:
                        nc.tensor.matmul(
                            ps_im[nt][:], ui[kt][:, nt * P:(nt + 1) * P], mt[kt][:],
                            start=(kt == 0), stop=(kt == KT - 1))
                if h == 0:
                    for nt in nts:
                        mk_sq_act(ps_im[nt], nt, 1)
            # tail: split the last half's squares across ACT and DVE
            mk_sq_act(ps_im[4], 4, 1)
            mk_sq_dve(ps_im[6], 6)
            mk_sq_act(ps_im[5], 5, 1)
            mk_sq_dve(ps_im[7], 7)

            ot = opool.tile([4, BC], f32, name="ot", tag="ot")
            nc.vector.tensor_copy(ot[:], ps_out[:])
            nc.sync.dma_start(out_d[:, :], ot[:])

    nc.finalize()
    _NC_CACHE[key] = nc
    return nc


LAST_RESULTS = None  # BassKernelResults of the most recent run (for test.py)


def kernel(**inputs):
    from concourse.bass_utils import run_bass_kernel_spmd

    global LAST_RESULTS
    x = np.asarray(inputs["inputs"], np.float32)
    weights = np.asarray(inputs["weights"], np.float32)
    fc_w = np.asarray(inputs["fc_w"], np.float32)
    fc_b = np.asarray(inputs["fc_b"], np.float32)

    m, UrT, UiT, W2 = _host_factors(x, weights, fc_w)
    w2pack = np.ascontiguousarray(
        W2.reshape(NT, P, 4).transpose(1, 0, 2).reshape(P, NT * 4))

    key = os.environ.get("KERNEL_MM_DTYPE", "fp16")
    mm_np = {"f32r": np.float32, "f32": np.float32,
             "fp16": np.float16, "bf16": None}[key]
    if mm_np is None:
        import ml_dtypes
        mm_np = ml_dtypes.bfloat16
    UrT = UrT.astype(mm_np)
    UiT = UiT.astype(mm_np)
    m_mm = m.astype(mm_np)
    if key in ("fp16", "bf16"):
        w2pack = w2pack.astype(mm_np)

    in_maps = []
    for c in range(N_CORES):
        mt_c = np.ascontiguousarray(m_mm[c * BC:(c + 1) * BC, :].T)
        in_maps.append({"mt": mt_c, "urt": UrT, "uit": UiT, "w2": w2pack})

    nc = _build_bass()
    trace = os.environ.get("KERNEL_TRACE", "0") == "1"
    res = run_bass_kernel_spmd(nc, in_maps, core_ids=list(range(N_CORES)),
                               trace=trace)
    LAST_RESULTS = res
    out = np.concatenate([r["outT"].T for r in res.results], axis=0)
    return (out + fc_b[None, :]).astype(np.float32)


# revision 16
# speedup vs baseline: 1.2883x; 1.2883x over previous
"""Trainium2 Bass kernel for nn_Net_81707457839588.

10-qubit batched statevector circuit + FC head, factorized as:
  psi_in  = m * phase          (product state after RX embedding; m real >=0... real)
  psi_out = U' @ m             (U' = circuit unitary with phases folded, shared
                                across batch since circuit weights are shared)
  out     = (re^2 + im^2) @ W2 + fc_b    (W2 = SIGNS @ fc_w.T folded)

Device work per core (512 of 4096 samples, pure batch-parallel):
  reT/imT (1024n x 512b) = sum_k U'T[k,n] * mT[k,b]   -> 128 PE matmuls (f32r)
  probsT = reT^2 + imT^2                              -> DVE
  outT (4 x 512) = W2.T @ probsT                      -> 8 PE matmuls
Host: builds U' (tiny circuit, 210 two-level gates on 1024x1024), m (kron
chain), shards batch across 8 cores, gathers, adds fc_b.
"""

import os
import numpy as np

N = 10
L = 10
BATCH = 4096
N_CORES = 8
BC = BATCH // N_CORES  # 512 samples per core
DIM = 2 ** N  # 1024
P = 128
KT = DIM // P  # 8 contraction tiles
NT = DIM // P  # 8 output n tiles


# ---------------------------------------------------------------- host math --

def _apply_gate_np(state, g, w):
    st = np.moveaxis(state, w + 1, 1)
    shp = st.shape
    out = np.einsum('ij,bjm->bim', g, st.reshape(shp[0], 2, -1)).reshape(shp)
    return np.moveaxis(out, 1, w + 1)


def _cnot_np(state, c, t):
    s0 = np.take(state, 0, axis=c + 1)
    s1 = np.take(state, 1, axis=c + 1)
    t_ax = t + 1 if t < c else t
    s1 = np.flip(s1, axis=t_ax)
    return np.stack([s0, s1], axis=c + 1)


def _build_UT(weights):
    """UT[j,k] = <k|U|j> for the StronglyEntanglingLayers circuit (no embedding)."""
    state = np.zeros((DIM, DIM), dtype=np.complex64)
    state[np.arange(DIM), np.arange(DIM)] = 1.0
    state = state.reshape((DIM,) + (2,) * N)
    for l in range(L):
        phi, theta, omega = weights[l, :, 0], weights[l, :, 1], weights[l, :, 2]
        ct, st_ = np.cos(0.5 * theta), np.sin(0.5 * theta)
        ep = np.exp(-0.5j * (phi + omega))
        em = np.exp(0.5j * (phi - omega))
        for w in range(N):
            g = np.array([
                [ep[w] * ct[w], -em[w] * st_[w]],
                [np.conj(em[w]) * st_[w], np.conj(ep[w]) * ct[w]],
            ], dtype=np.complex64)
            state = _apply_gate_np(state, g, w)
        r = l % (N - 1) + 1
        for w in range(N):
            state = _cnot_np(state, w, (w + r) % N)
    return np.ascontiguousarray(state.reshape(DIM, DIM))


def _pauli_z_signs():
    idx = np.arange(DIM)
    bits = (idx[:, None] >> (N - 1 - np.arange(N))) & 1
    return (1.0 - 2.0 * bits).astype(np.float32)


def _host_factors(inputs, weights, fc_w):
    x = np.maximum(np.asarray(inputs, np.float32), 0.0)
    half = 0.5 * x
    c, s = np.cos(half), np.sin(half)
    B = x.shape[0]
    m = np.ones((B, 1), dtype=np.float32)
    for w in range(N):
        v = np.stack([c[:, w], s[:, w]], axis=-1)
        m = (m[:, :, None] * v[:, None, :]).reshape(B, -1)
    k = np.arange(DIM)
    pc = np.zeros(DIM, dtype=np.int64)
    for w in range(N):
        pc += (k >> w) & 1
    ph = ((-1j) ** (pc % 4)).astype(np.complex64)

    UpT = _build_UT(np.asarray(weights, np.float32)) * ph[:, None]
    UrT = np.ascontiguousarray(UpT.real.astype(np.float32))
    UiT = np.ascontiguousarray(UpT.imag.astype(np.float32))
    W2 = _pauli_z_signs() @ np.asarray(fc_w, np.float32).T  # (1024, 4)
    return m, UrT, UiT, W2


# ------------------------------------------------------------- bass kernel --

_NC_CACHE = {}


def _build_bass():
    key = os.environ.get("KERNEL_MM_DTYPE", "fp16")
    if key in _NC_CACHE:
        return _NC_CACHE[key]
    import concourse.bacc as bacc
    import concourse.mybir as mybir
    from concourse.tile import TileContext

    f32 = mybir.dt.float32
    mm_dt = {"f32r": mybir.dt.float32r, "f32": mybir.dt.float32,
             "fp16": mybir.dt.float16, "bf16": mybir.dt.bfloat16}[key]
    small_dt = mm_dt if key in ("fp16", "bf16") else f32

    nc = bacc.Bacc()
    mt_d = nc.dram_tensor("mt", (DIM, BC), mm_dt, kind="ExternalInput")
    urt_d = nc.dram_tensor("urt", (DIM, DIM), mm_dt, kind="ExternalInput")
    uit_d = nc.dram_tensor("uit", (DIM, DIM), mm_dt, kind="ExternalInput")
    w2_d = nc.dram_tensor("w2", (P, NT * 4), small_dt, kind="ExternalInput")
    out_d = nc.dram_tensor("outT", (4, BC), f32, kind="ExternalOutput")

    with TileContext(nc) as tc:
        with tc.tile_pool(name="wts", bufs=1) as wpool, \
             tc.tile_pool(name="acts", bufs=1) as apool, \
             tc.tile_pool(name="sq", bufs=1) as sqpool, \
             tc.tile_pool(name="tmp", bufs=4) as tpool, \
             tc.tile_pool(name="ps", bufs=8, space="PSUM") as pspool, \
             tc.tile_pool(name="outp", bufs=1) as opool:

            # ---- PE warm-up on dummy data: HAM un-throttles after ~3.4us of
            # sustained PE activity, so burn that window before real data lands.
            dummy = apool.tile([P, 256], mm_dt, name="dummy", tag="dummy")
            nc.vector.memset(dummy[:], 0)
            ps_w = pspool.tile([P, 256], f32, name="psw", tag="ps")
            for i in range(14):
                nc.tensor.matmul(ps_w[:], dummy[:, 0:P], dummy[:],
                                 start=(i == 0), stop=(i == 13))
            scrap = opool.tile([P, 8], f32, name="scrap", tag="scrap")
            nc.vector.tensor_copy(scrap[:], ps_w[:, 0:8])

            # ---- loads: all on sync; serialized triggers pace ui after mt/ur
            mt, ur, ui = [], [], []
            for kt in range(KT):
                t = apool.tile([P, BC], mm_dt, name=f"mt{kt}", tag=f"mt{kt}")
                nc.sync.dma_start(t[:], mt_d[kt * P:(kt + 1) * P, :])
                mt.append(t)
                t = wpool.tile([P, DIM], mm_dt, name=f"ur{kt}", tag=f"ur{kt}")
                nc.sync.dma_start(t[:], urt_d[kt * P:(kt + 1) * P, :])
                ur.append(t)
            w2t = wpool.tile([P, NT * 4], small_dt, name="w2t", tag="w2")
            nc.sync.dma_start(w2t[:], w2_d[:, :])
            for kt in range(KT):
                t = wpool.tile([P, DIM], mm_dt, name=f"ui{kt}", tag=f"ui{kt}")
                nc.sync.dma_start(t[:], uit_d[kt * P:(kt + 1) * P, :])
                ui.append(t)

            # ---- real part (kt-outer: PE rides the DMA stream)
            ps_re = [pspool.tile([P, BC], f32, name=f"psre{i}", tag="ps") for i in range(NT)]
            for kt in range(KT):
                for nt in range(NT):
                    nc.tensor.matmul(
                        ps_re[nt][:], ur[kt][:, nt * P:(nt + 1) * P], mt[kt][:],
                        start=(kt == 0), stop=(kt == KT - 1))
            sq_re = []
            for nt in range(NT):
                t = sqpool.tile([P, BC], f32, name=f"sqre{nt}", tag=f"sqre{nt}")
                nc.scalar.square(t[:], ps_re[nt][:])
                sq_re.append(t)

            # ---- imaginary part
            ps_im = [pspool.tile([P, BC], f32, name=f"psim{i}", tag="ps") for i in range(NT)]
            for kt in range(KT):
                for nt in range(NT):
                    nc.tensor.matmul(
                        ps_im[nt][:], ui[kt][:, nt * P:(nt + 1) * P], mt[kt][:],
                        start=(kt == 0), stop=(kt == KT - 1))
            probs = []
            for nt in range(NT):
                t2 = tpool.tile([P, BC], f32, name=f"sqim{nt}", tag="sqim")
                nc.scalar.square(t2[:], ps_im[nt][:])
                t3 = sqpool.tile([P, BC], small_dt, name=f"probs{nt}", tag=f"probs{nt}")
                nc.vector.tensor_add(t3[:], t2[:], sq_re[nt][:])
                probs.append(t3)

            # ---- projection: outT[a, b] = sum_n W2[n, a] * probsT[n, b]
            ps_out = pspool.tile([4, BC], f32, name="psout", tag="ps")
            for nt in range(NT):
                nc.tensor.matmul(
                    ps_out[:], w2t[:, nt * 4:(nt + 1) * 4], probs[nt][:],
                    start=(nt == 0), stop=(nt == NT - 1))
            ot = opool.tile([4, BC], f32, name="ot", tag="ot")
            nc.vector.tensor_copy(ot[:], ps_out[:])
            nc.sync.dma_start(out_d[:, :], ot[:])

    nc.finalize()
    _NC_CACHE[key] = nc
    return nc


LAST_RESULTS = None  # BassKernelResults of the most recent run (for test.py)


def kernel(**inputs):
    from concourse.bass_utils import run_bass_kernel_spmd

    global LAST_RESULTS
    x = np.asarray(inputs["inputs"], np.float32)
    weights = np.asarray(inputs["weights"], np.float32)
    fc_w = np.asarray(inputs["fc_w"], np.float32)
    fc_b = np.asarray(inputs["fc_b"], np.float32)

    m, UrT, UiT, W2 = _host_factors(x, weights, fc_w)
    w2pack = np.ascontiguousarray(
        W2.reshape(NT, P, 4).transpose(1, 0, 2).reshape(P, NT * 4))

    key = os.environ.get("KERNEL_MM_DTYPE", "fp16")
    mm_np = {"f32r": np.float32, "f32": np.float32,
             "fp16": np.float16, "bf16": None}[key]
    if mm_np is None:
        import ml_dtypes
        mm_np = ml_dtypes.bfloat16
    UrT = UrT.astype(mm_np)
    UiT = UiT.astype(mm_np)
    m_mm = m.astype(mm_np)
    if key in ("fp16", "bf16"):
        w2pack = w2pack.astype(mm_np)

    in_maps = []
    for c in range(N_CORES):
        mt_c = np.ascontiguousarray(m_mm[c * BC:(c + 1) * BC, :].T)
        in_maps.append({"mt": mt_c, "urt": UrT, "uit": UiT, "w2": w2pack})

    nc = _build_bass()
    trace = os.environ.get("KERNEL_TRACE", "0") == "1"
    res = run_bass_kernel_spmd(nc, in_maps, core_ids=list(range(N_CORES)),
                               trace=trace)
    LAST_RESULTS = res
    out = np.concatenate([r["outT"].T for r in res.results], axis=0)
    return (out + fc_b[None, :]).astype(np.float32)


# revision 17
# speedup vs baseline: 1.3177x; 1.0228x over previous
"""Trainium2 Bass kernel for nn_Net_81707457839588.

10-qubit batched statevector circuit + FC head, factorized as:
  psi_in  = m * phase          (product state after RX embedding; m real >=0... real)
  psi_out = U' @ m             (U' = circuit unitary with phases folded, shared
                                across batch since circuit weights are shared)
  out     = (re^2 + im^2) @ W2 + fc_b    (W2 = SIGNS @ fc_w.T folded)

Device work per core (512 of 4096 samples, pure batch-parallel):
  reT/imT (1024n x 512b) = sum_k U'T[k,n] * mT[k,b]   -> 128 PE matmuls (f32r)
  probsT = reT^2 + imT^2                              -> DVE
  outT (4 x 512) = W2.T @ probsT                      -> 8 PE matmuls
Host: builds U' (tiny circuit, 210 two-level gates on 1024x1024), m (kron
chain), shards batch across 8 cores, gathers, adds fc_b.
"""

import os
import numpy as np

N = 10
L = 10
BATCH = 4096
N_CORES = 8
BC = BATCH // N_CORES  # 512 samples per core
DIM = 2 ** N  # 1024
P = 128
KT = DIM // P  # 8 contraction tiles
NT = DIM // P  # 8 output n tiles


# ---------------------------------------------------------------- host math --

def _apply_gate_np(state, g, w):
    st = np.moveaxis(state, w + 1, 1)
    shp = st.shape
    out = np.einsum('ij,bjm->bim', g, st.reshape(shp[0], 2, -1)).reshape(shp)
    return np.moveaxis(out, 1, w + 1)


def _cnot_np(state, c, t):
    s0 = np.take(state, 0, axis=c + 1)
    s1 = np.take(state, 1, axis=c + 1)
    t_ax = t + 1 if t < c else t
    s1 = np.flip(s1, axis=t_ax)
    return np.stack([s0, s1], axis=c + 1)


def _build_UT(weights):
    """UT[j,k] = <k|U|j> for the StronglyEntanglingLayers circuit (no embedding)."""
    state = np.zeros((DIM, DIM), dtype=np.complex64)
    state[np.arange(DIM), np.arange(DIM)] = 1.0
    state = state.reshape((DIM,) + (2,) * N)
    for l in range(L):
        phi, theta, omega = weights[l, :, 0], weights[l, :, 1], weights[l, :, 2]
        ct, st_ = np.cos(0.5 * theta), np.sin(0.5 * theta)
        ep = np.exp(-0.5j * (phi + omega))
        em = np.exp(0.5j * (phi - omega))
        for w in range(N):
            g = np.array([
                [ep[w] * ct[w], -em[w] * st_[w]],
                [np.conj(em[w]) * st_[w], np.conj(ep[w]) * ct[w]],
            ], dtype=np.complex64)
            state = _apply_gate_np(state, g, w)
        r = l % (N - 1) + 1
        for w in range(N):
            state = _cnot_np(state, w, (w + r) % N)
    return np.ascontiguousarray(state.reshape(DIM, DIM))


def _pauli_z_signs():
    idx = np.arange(DIM)
    bits = (idx[:, None] >> (N - 1 - np.arange(N))) & 1
    return (1.0 - 2.0 * bits).astype(np.float32)


def _host_factors(inputs, weights, fc_w):
    x = np.maximum(np.asarray(inputs, np.float32), 0.0)
    half = 0.5 * x
    c, s = np.cos(half), np.sin(half)
    B = x.shape[0]
    m = np.ones((B, 1), dtype=np.float32)
    for w in range(N):
        v = np.stack([c[:, w], s[:, w]], axis=-1)
        m = (m[:, :, None] * v[:, None, :]).reshape(B, -1)
    k = np.arange(DIM)
    pc = np.zeros(DIM, dtype=np.int64)
    for w in range(N):
        pc += (k >> w) & 1
    ph = ((-1j) ** (pc % 4)).astype(np.complex64)

    UpT = _build_UT(np.asarray(weights, np.float32)) * ph[:, None]
    UrT = np.ascontiguousarray(UpT.real.astype(np.float32))
    UiT = np.ascontiguousarray(UpT.imag.astype(np.float32))
    W2 = _pauli_z_signs() @ np.asarray(fc_w, np.float32).T  # (1024, 4)
    return m, UrT, UiT, W2


# ------------------------------------------------------------- bass kernel --

_NC_CACHE = {}


def _build_bass():
    key = os.environ.get("KERNEL_MM_DTYPE", "fp16")
    if key in _NC_CACHE:
        return _NC_CACHE[key]
    import concourse.bacc as bacc
    import concourse.mybir as mybir
    from concourse.tile import TileContext

    f32 = mybir.dt.float32
    mm_dt = {"f32r": mybir.dt.float32r, "f32": mybir.dt.float32,
             "fp16": mybir.dt.float16, "bf16": mybir.dt.bfloat16}[key]
    small_dt = mm_dt if key in ("fp16", "bf16") else f32

    nc = bacc.Bacc()
    mt_d = nc.dram_tensor("mt", (DIM, BC), mm_dt, kind="ExternalInput")
    urt_d = nc.dram_tensor("urt", (DIM, DIM), mm_dt, kind="ExternalInput")
    uit_d = nc.dram_tensor("uit", (DIM, DIM), mm_dt, kind="ExternalInput")
    w2_d = nc.dram_tensor("w2", (P, NT * 4), small_dt, kind="ExternalInput")
    out_d = nc.dram_tensor("outT", (4, BC), f32, kind="ExternalOutput")

    with TileContext(nc) as tc:
        with tc.tile_pool(name="wts", bufs=1) as wpool, \
             tc.tile_pool(name="acts", bufs=1) as apool, \
             tc.tile_pool(name="sq", bufs=1) as sqpool, \
             tc.tile_pool(name="tmp", bufs=4) as tpool, \
             tc.tile_pool(name="ps", bufs=8, space="PSUM") as pspool, \
             tc.tile_pool(name="outp", bufs=1) as opool:

            # ---- PE warm-up on dummy data: HAM un-throttles after ~3.4us of
            # sustained PE activity, so burn that window before real data lands.
            dummy = apool.tile([P, 256], mm_dt, name="dummy", tag="dummy")
            nc.vector.memset(dummy[:], 0)
            ps_w = pspool.tile([P, 256], f32, name="psw", tag="ps")
            for i in range(14):
                nc.tensor.matmul(ps_w[:], dummy[:, 0:P], dummy[:],
                                 start=(i == 0), stop=(i == 13))
            scrap = opool.tile([P, 8], f32, name="scrap", tag="scrap")
            nc.vector.tensor_copy(scrap[:], ps_w[:, 0:8])

            # ---- loads: all on sync; serialized triggers pace ui after mt/ur
            mt, ur, ui = [], [], []
            for kt in range(KT):
                t = apool.tile([P, BC], mm_dt, name=f"mt{kt}", tag=f"mt{kt}")
                nc.sync.dma_start(t[:], mt_d[kt * P:(kt + 1) * P, :])
                mt.append(t)
                t = wpool.tile([P, DIM], mm_dt, name=f"ur{kt}", tag=f"ur{kt}")
                nc.sync.dma_start(t[:], urt_d[kt * P:(kt + 1) * P, :])
                ur.append(t)
            w2t = wpool.tile([P, NT * 4], small_dt, name="w2t", tag="w2")
            nc.sync.dma_start(w2t[:], w2_d[:, :])
            for kt in range(KT):
                t = wpool.tile([P, DIM], mm_dt, name=f"ui{kt}", tag=f"ui{kt}")
                nc.sync.dma_start(t[:], uit_d[kt * P:(kt + 1) * P, :])
                ui.append(t)

            # ---- real part (kt-outer: PE rides the DMA stream)
            ps_re = [pspool.tile([P, BC], f32, name=f"psre{i}", tag="ps") for i in range(NT)]
            for kt in range(KT):
                for nt in range(NT):
                    nc.tensor.matmul(
                        ps_re[nt][:], ur[kt][:, nt * P:(nt + 1) * P], mt[kt][:],
                        start=(kt == 0), stop=(kt == KT - 1))
            sq_re = []
            for nt in range(NT):
                t = sqpool.tile([P, BC], small_dt, name=f"sqre{nt}", tag=f"sqre{nt}")
                nc.scalar.square(t[:], ps_re[nt][:])
                sq_re.append(t)

            # ---- imaginary part, two 4-bank halves: the first half's
            # squares/adds hide under the second half's matmuls
            probs = [None] * NT
            for h in range(2):
                nts = list(range(h * (NT // 2), (h + 1) * (NT // 2)))
                ps_im = {nt: pspool.tile([P, BC], f32, name=f"psim{nt}", tag="ps")
                         for nt in nts}
                for kt in range(KT):
                    for nt in nts:
                        nc.tensor.matmul(
                            ps_im[nt][:], ui[kt][:, nt * P:(nt + 1) * P], mt[kt][:],
                            start=(kt == 0), stop=(kt == KT - 1))
                for nt in nts:
                    t2 = tpool.tile([P, BC], small_dt, name=f"sqim{nt}", tag="sqim")
                    nc.scalar.square(t2[:], ps_im[nt][:])
                    t3 = sqpool.tile([P, BC], small_dt, name=f"probs{nt}", tag=f"probs{nt}")
                    nc.vector.tensor_add(t3[:], t2[:], sq_re[nt][:])
                    probs[nt] = t3

            # ---- projection: outT[a, b] = sum_n W2[n, a] * probsT[n, b]
            ps_out = pspool.tile([4, BC], f32, name="psout", tag="ps")
            for nt in range(NT):
                nc.tensor.matmul(
                    ps_out[:], w2t[:, nt * 4:(nt + 1) * 4], probs[nt][:],
                    start=(nt == 0), stop=(nt == NT - 1))
            ot = opool.tile([4, BC], f32, name="ot", tag="ot")
            nc.vector.tensor_copy(ot[:], ps_out[:])
            nc.sync.dma_start(out_d[:, :], ot[:])

    nc.finalize()
    _NC_CACHE[key] = nc
    return nc


LAST_RESULTS = None  # BassKernelResults of the most recent run (for test.py)


def kernel(**inputs):
    from concourse.bass_utils import run_bass_kernel_spmd

    global LAST_RESULTS
    x = np.asarray(inputs["inputs"], np.float32)
    weights = np.asarray(inputs["weights"], np.float32)
    fc_w = np.asarray(inputs["fc_w"], np.float32)
    fc_b = np.asarray(inputs["fc_b"], np.float32)

    m, UrT, UiT, W2 = _host_factors(x, weights, fc_w)
    w2pack = np.ascontiguousarray(
        W2.reshape(NT, P, 4).transpose(1, 0, 2).reshape(P, NT * 4))

    key = os.environ.get("KERNEL_MM_DTYPE", "fp16")
    mm_np = {"f32r": np.float32, "f32": np.float32,
             "fp16": np.float16, "bf16": None}[key]
    if mm_np is None:
        import ml_dtypes
        mm_np = ml_dtypes.bfloat16
    UrT = UrT.astype(mm_np)
    UiT = UiT.astype(mm_np)
    m_mm = m.astype(mm_np)
    if key in ("fp16", "bf16"):
        w2pack = w2pack.astype(mm_np)

    in_maps = []
    for c in range(N_CORES):
        mt_c = np.ascontiguousarray(m_mm[c * BC:(c + 1) * BC, :].T)
        in_maps.append({"mt": mt_c, "urt": UrT, "uit": UiT, "w2": w2pack})

    nc = _build_bass()
    trace = os.environ.get("KERNEL_TRACE", "0") == "1"
    res = run_bass_kernel_spmd(nc, in_maps, core_ids=list(range(N_CORES)),
                               trace=trace)
    LAST_RESULTS = res
    out = np.concatenate([r["outT"].T for r in res.results], axis=0)
    return (out + fc_b[None, :]).astype(np.float32)
